# revision 1
# baseline (speedup 1.0000x reference)
"""Trainium2 Bass kernel for AdaptiveLRLinearWithChannel (moe_routing).

Reference math:
    w    = (weights_U[indices] @ weights_V).reshape(B, IN, OUT)
    out  = einsum('bni,bio->bno', x, w) + bias[indices]

Strategy (8 NeuronCores, data-parallel over B):
  - Shard B=256 into 8 x 32 batches. Host marshalling only: gather U[idx] /
    bias[idx], re-lay x out as xT[b] = x[b].T so the contraction dim (IN)
    lands on SBUF partitions, cast to bf16 (matmul accumulates in f32 PSUM;
    rel err ~3e-3). All O(B*N*IN*OUT) FLOPs and the low-rank weight
    synthesis w[b] = sum_r U[b,r] * V_r run on device: VectorE fma's the 4
    rank-1 components into each per-batch weight tile (saves 4MB/core of
    DMA vs shipping W; the kernel is HBM-bandwidth-bound).
  - The kernel computes outT[b] = W[b].T @ x[b].T tile-by-tile: the stationary
    operand is a W chunk (reused across 4 moving passes of 512 columns of
    xT), so the output lands transposed ([o, n]) in PSUM. That puts the bias
    on the partition axis, so PSUM evacuation (+bias, ->bf16) is split
    between VectorE (tensor_scalar_add) and ScalarE (activation Copy+bias),
    and the bias input is 32KB instead of MBs.
  - Outputs are stored partition-major ([b, o-chunk, o_lane, n]); the host
    un-permutes to [b, n, o]. Loads run on the SP HWDGE ring, stores on the
    ACT ring.
"""

import sys

for _p in ("/opt/trn_rl_repo",):
    if _p not in sys.path:
        sys.path.insert(0, _p)

import numpy as np

B = 256
N = 2048
IN_SZ = 256
OUT_SZ = 256
N_CORES = 8
BPC = B // N_CORES  # 32 batches per core
NSL = 4  # moving n-slices per (batch, o-chunk); each 512 wide
NSW = N // NSL  # 512
XBUFS = 12
OBUFS = 14
PBUFS = 8
STAGGERED = True  # staggered_reset on the timing For_i loop

_CACHE = {}


def _bf16():
    import ml_dtypes

    return ml_dtypes.bfloat16


def _emit_body(nc, xT, vdev_sb, ubc_sb, out, bias_sb, xp, wp, op, psum):
    import concourse.mybir as mybir

    bf16 = mybir.dt.bfloat16
    f32 = mybir.dt.float32
    Copy = mybir.ActivationFunctionType.Identity

    # On-device low-rank W synthesis: w[b, 128*ci+p, o] = sum_r U[b,r] *
    # V[r, (128*ci+p)*256+o].  vdev_sb[p, r*512 + ci*256 + o] holds V so one
    # 512-wide pass per rank covers both K-chunks; ubc_sb[p, b*4+r] holds
    # U[b, r] replicated across partitions.  The r=0 scale-multiply runs on
    # ScalarE (activation Identity with per-partition scale); ranks 1-3 are
    # VectorE fma passes, the last writing the bf16 wt tile.
    mult = mybir.AluOpType.mult
    add = mybir.AluOpType.add
    W2 = 2 * OUT_SZ

    for b in range(BPC):
        wt_b = wp.tile([128, W2], bf16, tag="wt", name=f"wt{b}")
        acc = wp.tile([128, W2], f32, tag="acc", name=f"acc{b}")
        nc.scalar.activation(
            acc[:],
            vdev_sb[:, 0:W2],
            Copy,
            scale=ubc_sb[:, b * 4 : b * 4 + 1],
        )
        for r in range(1, 4):
            vsl = vdev_sb[:, r * W2 : (r + 1) * W2]
            usl = ubc_sb[:, b * 4 + r : b * 4 + r + 1]
            dst = acc[:] if r < 3 else wt_b[:]
            nc.vector.scalar_tensor_tensor(dst, vsl, usl, acc[:], mult, add)
        # xt[p, ci*N + n] = xT[b, 128*ci + p, n]; two fully-contiguous
        # 512KB reads (one per K-chunk)
        xt = xp.tile([128, 2 * N], bf16, tag="xt")
        nc.sync.dma_start(out=xt[:, 0:N], in_=xT[b, 0:128, :])
        nc.sync.dma_start(out=xt[:, N : 2 * N], in_=xT[b, 128:256, :])
        for co in range(2):
            og = op.tile([128, N], bf16, tag="og")
            pss = [
                psum.tile([128, NSW], f32, tag="ps", name=f"ps{s}")
                for s in range(NSL)
            ]
            for ci in range(2):
                base = ci * OUT_SZ + co * 128
                lhsT = wt_b[:, base : base + 128]
                for s in range(NSL):
                    nc.tensor.matmul(
                        pss[s][:],
                        lhsT=lhsT,
                        rhs=xt[:, ci * N + s * NSW : ci * N + (s + 1) * NSW],
                        start=(ci == 0),
                        stop=(ci == 1),
                    )
            bias_col = bias_sb[:, b * 2 + co : b * 2 + co + 1]  # [128, 1] f32
            for s in range(NSL):
                dst = og[:, s * NSW : (s + 1) * NSW]
                if s == 0:
                    nc.vector.tensor_scalar_add(dst, pss[s][:], bias_col)
                else:
                    nc.scalar.activation(dst, pss[s][:], Copy, bias=bias_col)
            # store [o_lane=128, n=2048] bf16 on the ACT HWDGE ring;
            # 4KB contiguous per partition. Host un-permutes.
            nc.scalar.dma_start(out=out[b, co], in_=og[:])


def build_nc(niter=1):
    """Build + compile the per-core Bass graph (same graph on all 8 cores).

    niter > 1 wraps the workload in an on-device For_i loop — used only for
    timing (amortizes host/tunnel dispatch overhead over many repeats).
    """
    key = ("nc", niter)
    if key in _CACHE:
        return _CACHE[key]

    import contextlib

    import concourse.mybir as mybir
    import concourse.tile as tile
    from concourse import bacc

    nc = bacc.Bacc("TRN2", target_bir_lowering=False, debug=False)
    bf16 = mybir.dt.bfloat16
    f32 = mybir.dt.float32

    xT = nc.declare_dram_parameter("xT", [BPC, IN_SZ, N], bf16, isOutput=False)
    vdev = nc.declare_dram_parameter("vdev", [128, 2 * 4 * OUT_SZ], bf16, isOutput=False)
    ubc = nc.declare_dram_parameter("ubc", [128, BPC * 4], f32, isOutput=False)
    # biasb[p, b*2+co] = bias_sel[b, co*128 + p]
    biasb = nc.declare_dram_parameter("biasb", [128, BPC * 2], f32, isOutput=False)
    # partition-major transposed output: out[b, co, p, n] = result[b, n, co*128+p]
    out = nc.declare_dram_parameter("out", [BPC, 2, 128, N], bf16, isOutput=True)
    nit = (
        nc.declare_dram_parameter("nit", [1, 1], mybir.dt.int32, isOutput=False)
        if niter == "dyn"
        else None
    )

    with tile.TileContext(nc) as tc:
        with (
            tc.tile_pool(name="bias", bufs=1) as biasp,
            tc.tile_pool(name="xp", bufs=XBUFS) as xp,
            tc.tile_pool(name="wp", bufs=10) as wp,
            tc.tile_pool(name="op", bufs=OBUFS) as op,
            tc.tile_pool(name="psum", bufs=PBUFS, space="PSUM") as psum,
        ):
            bias_sb = biasp.tile([128, BPC * 2], f32, tag="bias")
            nc.sync.dma_start(out=bias_sb[:], in_=biasb[:])
            vdev_sb = biasp.tile([128, 2 * 4 * OUT_SZ], bf16, tag="vdev")
            nc.sync.dma_start(out=vdev_sb[:], in_=vdev[:])
            ubc_sb = biasp.tile([128, BPC * 4], f32, tag="ubc")
            nc.sync.dma_start(out=ubc_sb[:], in_=ubc[:])

            if niter == "dyn":
                nit_tile = biasp.tile([1, 1], mybir.dt.int32, tag="nit")
                nc.sync.dma_start(out=nit_tile[:], in_=nit[:])
                nval = nc.values_load(
                    nit_tile[0:1, 0:1],
                    min_val=1,
                    max_val=1 << 20,
                    skip_runtime_bounds_check=True,
                )
                ctx = tc.For_i(0, nval, 1, staggered_reset=STAGGERED)
            elif niter > 1:
                ctx = tc.For_i(0, niter, 1, staggered_reset=STAGGERED)
            else:
                ctx = contextlib.nullcontext()
            with ctx:
                _emit_body(nc, xT, vdev_sb, ubc_sb, out, bias_sb, xp, wp, op, psum)

    nc.compile()
    _CACHE[key] = nc
    return nc


def prep_in_maps(x, indices, weights_U, weights_V, bias):
    """Host-side marshalling: gather/synthesize per-batch weights, transpose
    x per batch, cast to bf16, shard along B."""
    bf16 = _bf16()
    x = np.asarray(x)
    idx = np.asarray(indices).astype(np.int64)
    U = np.asarray(weights_U, dtype=np.float32)
    V = np.asarray(weights_V, dtype=np.float32)
    bias = np.asarray(bias, dtype=np.float32)

    xT = np.ascontiguousarray(x.transpose(0, 2, 1)).astype(bf16)  # [B, in, n]
    bias_sel = bias[idx][:, 0, :]  # [B, out] f32
    U_sel = U[idx]  # [B, 4] f32
    # vdev[p, r*512 + ci*256 + o] = V[r, (128*ci+p)*256+o]
    V4 = V.reshape(4, 2, 128, OUT_SZ)  # [r, ci, p, o]
    vdev = np.ascontiguousarray(
        V4.transpose(2, 0, 1, 3).reshape(128, 2 * 4 * OUT_SZ)
    ).astype(bf16)

    in_maps = []
    for c in range(N_CORES):
        s = slice(c * BPC, (c + 1) * BPC)
        # [128, BPC*2]: biasb[p, b*2+co] = bias_sel[b, co*128+p]
        bias_pm = np.ascontiguousarray(
            bias_sel[s].reshape(BPC, 2, 128).transpose(2, 0, 1).reshape(128, BPC * 2),
            dtype=np.float32,
        )
        ubc = np.ascontiguousarray(
            np.broadcast_to(U_sel[s].reshape(1, BPC * 4), (128, BPC * 4)),
            dtype=np.float32,
        )
        in_maps.append({"xT": xT[s], "vdev": vdev, "ubc": ubc, "biasb": bias_pm})
    return in_maps


def assemble_output(results):
    out = np.concatenate(
        [np.asarray(results[c]["out"], dtype=np.float32) for c in range(N_CORES)],
        axis=0,
    )
    # [B, co, p, n] -> [B, n, o=co*128+p]
    out = out.transpose(0, 3, 1, 2).reshape(B, N, OUT_SZ)
    return np.ascontiguousarray(out)


def kernel(x, indices, weights_U, weights_V, bias):
    from concourse import bass2jax

    nc = build_nc()
    in_maps = prep_in_maps(x, indices, weights_U, weights_V, bias)
    results = bass2jax.run_bass_via_pjrt(nc, in_maps, n_cores=N_CORES)
    return assemble_output(results)

